# revision 1
# baseline (speedup 1.0000x reference)
"""BiLSTM-CRF NER loss kernel for 8 Trainium2 NeuronCores.

Strategy: data-parallel — 8 examples per core. Per core:
  P0  embedding gather (indirect DMA) + PE transpose -> xT [E-on-partitions] bf16
  P1  input projections u = x @ W_ih.T + b for both directions (big matmuls,
      padded gate layout: each 300-wide gate padded to 384 = 3x128 chunks)
  P2  fwd+bwd LSTM recurrences interleaved superstep-wise (hidden-on-partitions,
      W_hh stationary bf16 tiles; gates on ACT, cell update on DVE)
  P3  emission matmul -> emit.T [12 tags on partitions, 2048 tok] f32
  P4  gold path score via one-hot mask + transition-select matmul + ones-matmul
  P5  CRF partition function in p-space: p_{t+1} = (exp(trans-3).T @ p_t) * E_{t+1}
      with E = exp(emit) bulk-precomputed; two independent half-batch chains;
      multiplicative renormalization every 8 steps (log-offsets accumulated in
      Mrow, constant 3(S-1) shift restored at the end)
  P6  loss = log_z - gold -> DRAM [8]
"""
import sys
sys.path.insert(0, '/opt/trn_rl_repo/concourse')
sys.path.insert(0, '/opt/trn_rl_repo')
import numpy as np
import ml_dtypes

E = 300
H = 300
NT = 12
BC = 8          # batch per core
NCORES = 8

_cache = {}


def _bf16(x):
    return np.asarray(x).astype(ml_dtypes.bfloat16)


def _pack_w(W):
    """(1200,300) -> packed lhsT [128, 3*1536] bf16 (K-chunk c at cols 1536c)."""
    P = np.zeros((384, 1536), np.float32)
    for slot, g in enumerate((0, 1, 3, 2)):   # i, f, o, g  (tanh gate last)
        P[:300, 384 * slot:384 * slot + 300] = W[300 * g:300 * g + 300, :].T
    packed = np.zeros((128, 3 * 1536), np.float32)
    for c in range(3):
        packed[:, 1536 * c:1536 * (c + 1)] = P[128 * c:128 * (c + 1), :]
    return _bf16(packed)


def _pack_w_fp8(W, fp8_np):
    """Recurrence weights: x16 scale into float8_e4m3 (rescaled by 1/16 on device)."""
    P = np.zeros((384, 1536), np.float32)
    for slot, g in enumerate((0, 1, 3, 2)):
        P[:300, 384 * slot:384 * slot + 300] = W[300 * g:300 * g + 300, :].T
    packed = np.zeros((128, 3 * 1536), np.float32)
    for c in range(3):
        packed[:, 1536 * c:1536 * (c + 1)] = P[128 * c:128 * (c + 1), :]
    return (packed * 16.0).astype(fp8_np)


def _pack_b(b):
    bp = np.zeros(1536, np.float32)
    for slot, g in enumerate((0, 1, 3, 2)):
        bp[384 * slot:384 * slot + 300] = b[300 * g:300 * g + 300]
    return np.ascontiguousarray(bp.reshape(12, 128).T, dtype=np.float32)


def _pack_lin(W_lin):
    P = np.zeros((768, 12), np.float32)
    P[0:300, :] = W_lin[:, 0:300].T
    P[384:684, :] = W_lin[:, 300:600].T
    packed = np.zeros((128, 6 * 12), np.float32)
    for c in range(6):
        packed[:, 12 * c:12 * (c + 1)] = P[128 * c:128 * (c + 1), :]
    return _bf16(packed)


def build(S=256, skip=()):
    """Build + compile the bass program. Returns (nc, names)."""
    from concourse import bass, mybir, bacc
    import concourse.tile as tile
    from concourse.masks import make_identity

    T = S * BC
    NG = T // 128            # number of 128-token gather groups
    f32 = mybir.dt.float32
    bf = mybir.dt.bfloat16
    i32 = mybir.dt.int32

    nc = bacc.Bacc("TRN2", target_bir_lowering=False, debug=False)
    names = {}
    with tile.TileContext(nc) as tc:
        with tc.tile_pool(name="dram", bufs=1, space="DRAM") as dram:
            d_sent = dram.tile([T], i32, kind="ExternalInput", name="sent")
            d_tags = dram.tile([T], i32, kind="ExternalInput", name="tags")
            d_embed = dram.tile([50000, E], f32, kind="ExternalInput", name="embed")
            d_pih_f = dram.tile([128, 4608], bf, kind="ExternalInput", name="pih_f")
            d_phh_f = dram.tile([128, 4608], mybir.dt.float8e4, kind="ExternalInput", name="phh_f")
            d_pih_b = dram.tile([128, 4608], bf, kind="ExternalInput", name="pih_b")
            d_phh_b = dram.tile([128, 4608], mybir.dt.float8e4, kind="ExternalInput", name="phh_b")
            d_bcol_f = dram.tile([128, 12], f32, kind="ExternalInput", name="bcol_f")
            d_bcol_b = dram.tile([128, 12], f32, kind="ExternalInput", name="bcol_b")
            d_plin = dram.tile([128, 72], bf, kind="ExternalInput", name="plin")
            d_blin = dram.tile([12, 1], f32, kind="ExternalInput", name="blin")
            d_trans = dram.tile([12, 12], f32, kind="ExternalInput", name="trans")
            d_transT = dram.tile([12, 12], f32, kind="ExternalInput", name="transT")
            d_loss = dram.tile([8, 1], f32, kind="ExternalOutput", name="loss")
            for k, v in [("sent", d_sent), ("tags", d_tags), ("embed", d_embed),
                         ("pih_f", d_pih_f), ("phh_f", d_phh_f), ("pih_b", d_pih_b),
                         ("phh_b", d_phh_b), ("bcol_f", d_bcol_f), ("bcol_b", d_bcol_b),
                         ("plin", d_plin), ("blin", d_blin), ("trans", d_trans),
                         ("transT", d_transT), ("loss", d_loss)]:
                names[k] = v.name

            with tc.tile_pool(name="const", bufs=1) as cp:
                ident = cp.tile([128, 128], f32)
                make_identity(nc, ident[:])
                pih = {"f": cp.tile([128, 4608], bf, name="pih_f_sb"), "b": cp.tile([128, 4608], bf, name="pih_b_sb")}
                phh = {"f": cp.tile([128, 4608], mybir.dt.float8e4, name="phh_f_sb"),
                       "b": cp.tile([128, 4608], mybir.dt.float8e4, name="phh_b_sb")}
                bcol = {"f": cp.tile([128, 12], f32, name="bcol_f_sb"), "b": cp.tile([128, 12], f32, name="bcol_b_sb")}
                plin = cp.tile([128, 72], bf)
                blin = cp.tile([12, 1], f32)
                trans_sb = cp.tile([12, 12], f32)
                transT_sb = cp.tile([12, 12], f32)
                texp = cp.tile([12, 12], f32)
                ones12 = cp.tile([12, 1], f32)
                iota_f = cp.tile([12, 1], f32)
                eps_b = cp.tile([12, 1], f32)
                nc.vector.memset(eps_b[:], 1e-30)
                negc = cp.tile([12, 1], f32)
                nc.vector.memset(negc[:], -3.0)
                nc.sync.dma_start(out=pih["f"][:], in_=d_pih_f[:])
                nc.sync.dma_start(out=phh["f"][:], in_=d_phh_f[:])
                nc.sync.dma_start(out=pih["b"][:], in_=d_pih_b[:])
                nc.sync.dma_start(out=phh["b"][:], in_=d_phh_b[:])
                nc.sync.dma_start(out=bcol["f"][:], in_=d_bcol_f[:])
                nc.sync.dma_start(out=bcol["b"][:], in_=d_bcol_b[:])
                nc.sync.dma_start(out=plin[:], in_=d_plin[:])
                nc.sync.dma_start(out=blin[:], in_=d_blin[:])
                nc.sync.dma_start(out=trans_sb[:], in_=d_trans[:])
                nc.sync.dma_start(out=transT_sb[:], in_=d_transT[:])
                nc.scalar.activation(out=texp[:], in_=trans_sb[:],
                                     func=mybir.ActivationFunctionType.Exp,
                                     bias=negc[:, 0:1])
                nc.vector.memset(ones12[:], 1.0)
                with tc.tile_pool(name="iota_tmp", bufs=1) as itp:
                    iota_i = itp.tile([12, 1], i32)
                    nc.gpsimd.iota(out=iota_i[:], pattern=[[0, 1]], base=0,
                                   channel_multiplier=1)
                    nc.vector.tensor_copy(out=iota_f[:], in_=iota_i[:])

                # big persistent tensors
                u = {"f": cp.tile([128, 12 * T], bf, name="u_f_sb"), "b": cp.tile([128, 12 * T], bf, name="u_b_sb")}
                hh = {"f": cp.tile([128, 3 * T], bf, name="hh_f_sb"), "b": cp.tile([128, 3 * T], bf, name="hh_b_sb")}
                emit = cp.tile([12, T], f32)
                mask = cp.tile([12, T + 8], f32)
                goldT = cp.tile([1, 8], f32)
                Mrow = cp.tile([1, 8], f32)
                D = cp.tile([12, 8], f32)
                loss_sb = cp.tile([8, 1], f32)

                # ---------------- P0: gather + transpose ----------------
                xtp_cm = tc.tile_pool(name="xtp", bufs=1)
                xtp = xtp_cm.__enter__()
                xT = xtp.tile([128, 3 * T], bf, name="xT_sb")
                nc.vector.memset(xT[:, 2 * T:3 * T], 0.0)
                with tc.tile_pool(name="p0", bufs=4) as p0, \
                     tc.tile_pool(name="p0ps", bufs=4, space="PSUM") as p0ps:
                  if "p0" not in skip:
                    idx = p0.tile([128, NG], i32, tag="idx")
                    nc.sync.dma_start(
                        out=idx[:], in_=d_sent[:].rearrange("(g p) -> p g", p=128))
                    for g in range(NG):
                        xr = p0.tile([128, E], f32, tag="xr")
                        nc.gpsimd.indirect_dma_start(
                            out=xr[:], out_offset=None, in_=d_embed[:],
                            in_offset=bass.IndirectOffsetOnAxis(ap=idx[:, g:g + 1], axis=0))
                        for s, (lo, sz) in enumerate([(0, 128), (128, 128), (256, 44)]):
                            pt = p0ps.tile([128, 128], f32, tag="pt")
                            nc.tensor.transpose(out=pt[0:sz, :], in_=xr[:, lo:lo + sz],
                                                identity=ident[:])
                            nc.vector.tensor_copy(
                                out=xT[0:sz, T * s + 128 * g: T * s + 128 * (g + 1)],
                                in_=pt[0:sz, :])

                # ---------------- P1: input projections ----------------
                with tc.tile_pool(name="p1ps", bufs=4, space="PSUM") as p1ps:
                  if "p1" not in skip:
                    for d in ("f", "b"):
                        for m in range(12):
                            for n in range(0, T, 512):
                                nn_ = min(512, T - n)
                                pu = p1ps.tile([128, 512], f32, tag="pu")
                                for c in range(3):
                                    nc.tensor.matmul(
                                        out=pu[:, 0:nn_],
                                        lhsT=pih[d][:, 1536 * c + 128 * m:1536 * c + 128 * (m + 1)],
                                        rhs=xT[:, T * c + n:T * c + n + nn_],
                                        start=(c == 0), stop=(c == 2))
                                nc.vector.tensor_scalar(
                                    out=u[d][:, T * m + n:T * m + n + nn_],
                                    in0=pu[:, 0:nn_], scalar1=bcol[d][:, m:m + 1],
                                    scalar2=None, op0=mybir.AluOpType.add)

                xtp_cm.__exit__(None, None, None)

                # tags broadcast to 12 partitions + mask build
                with tc.tile_pool(name="ptg", bufs=1) as ptg:
                  if "ptg" not in skip:
                    tagsr = ptg.tile([12, T], i32, tag="tagsr")
                    for j in range(12):
                        nc.sync.dma_start(out=tagsr[j:j + 1, :],
                                          in_=d_tags[:].rearrange("(a t) -> a t", a=1))
                    tags_f = ptg.tile([12, T], f32, tag="tagsf")
                    nc.vector.tensor_copy(out=tags_f[:], in_=tagsr[:])
                    nc.vector.memset(mask[:, T:T + 8], 0.0)
                    nc.vector.tensor_scalar(
                        out=mask[:, 0:T], in0=tags_f[:], scalar1=iota_f[:, 0:1],
                        scalar2=None, op0=mybir.AluOpType.is_equal)

                # ---------------- P2: interleaved recurrences ----------------
                with tc.tile_pool(name="p2", bufs=4) as p2, \
                     tc.tile_pool(name="p2c", bufs=1) as p2c, \
                     tc.tile_pool(name="p2ps", bufs=4, space="PSUM") as p2ps:
                    cst = {d: p2c.tile([128, 24], f32, tag=f"c_{d}", name=f"cst_{d}") for d in "fb"}
                    h0 = p2c.tile([128, 24], bf, tag="h0")
                    nc.vector.memset(h0[:], 0.0)
                    for d in "fb":
                        nc.vector.memset(cst[d][:], 0.0)

                    def dir_mms(d, t, tprev):
                        # two psum halves: A = i,f chunks (m 0-5), B = o,g (m 6-11)
                        pgA = p2ps.tile([128, 48], f32, tag=f"pgA_{d}", name=f"pgA_{d}_{t}", bufs=2)
                        pgB = p2ps.tile([128, 48], f32, tag=f"pgB_{d}", name=f"pgB_{d}_{t}", bufs=2)
                        is_h0 = tprev is None or "norecur" in skip
                        rhs_all = h0 if is_h0 else hh[d]
                        roff = 0 if is_h0 else 8 * tprev
                        for m in range(12):
                            pg, mo = (pgA, m) if m < 6 else (pgB, m - 6)
                            for c in range(3):
                                rsl = (rhs_all[:, 8 * c:8 * c + 8] if is_h0 else
                                       rhs_all[:, T * c + roff:T * c + roff + 8])
                                nc.tensor.matmul(
                                    out=pg[:, 8 * mo:8 * (mo + 1)],
                                    lhsT=phh[d][:, 1536 * c + 128 * m:1536 * c + 128 * (m + 1)],
                                    rhs=rsl, start=(c == 0), stop=(c == 2))
                            if m == 5:
                                # i,f pre-acts + sigmoid overlap the o,g matmuls
                                gact = p2.tile([128, 96], f32, tag=f"gact_{d}",
                                               name=f"gact_{d}_{t}")
                                uslA = u[d][:].rearrange("p (m x) -> p m x", m=12)[:, 0:6, 8 * t:8 * t + 8]
                                nc.vector.scalar_tensor_tensor(
                                    out=gact[:, 0:48], in0=pgA[:], scalar=0.0625,
                                    in1=uslA, op0=mybir.AluOpType.mult,
                                    op1=mybir.AluOpType.add)
                                nc.scalar.activation(out=gact[:, 0:48], in_=gact[:, 0:48],
                                                     func=mybir.ActivationFunctionType.Sigmoid)
                        return gact, pgB

                    def dir_gates(d, t, packed):
                        gact, pgB = packed
                        gpre = p2.tile([128, 48], f32, tag=f"gpre_{d}")
                        uslB = u[d][:].rearrange("p (m x) -> p m x", m=12)[:, 6:12, 8 * t:8 * t + 8]
                        nc.vector.scalar_tensor_tensor(
                            out=gpre[:], in0=pgB[:], scalar=0.0625, in1=uslB,
                            op0=mybir.AluOpType.mult, op1=mybir.AluOpType.add)
                        nc.scalar.activation(out=gact[:, 48:72], in_=gpre[:, 0:24],
                                             func=mybir.ActivationFunctionType.Sigmoid)
                        nc.scalar.activation(out=gact[:, 72:96], in_=gpre[:, 24:48],
                                             func=mybir.ActivationFunctionType.Tanh)
                        ig = p2.tile([128, 24], f32, tag=f"ig_{d}")
                        nc.vector.tensor_mul(out=ig[:], in0=gact[:, 0:24], in1=gact[:, 72:96])
                        nc.vector.tensor_mul(out=cst[d][:], in0=gact[:, 24:48], in1=cst[d][:])
                        nc.vector.tensor_add(out=cst[d][:], in0=cst[d][:], in1=ig[:])
                        tc_t = p2.tile([128, 24], f32, tag=f"tc_{d}")
                        nc.scalar.activation(out=tc_t[:], in_=cst[d][:],
                                             func=mybir.ActivationFunctionType.Tanh)
                        hsl = hh[d][:].rearrange("p (c x) -> p c x", c=3)[:, :, 8 * t:8 * t + 8]
                        nc.vector.tensor_mul(out=hsl, in0=tc_t[:].rearrange("p (c x) -> p c x", c=3),
                                             in1=gact[:, 48:72].rearrange("p (c x) -> p c x", c=3))

                    if "p2" in skip:
                        for d in "fb":
                            nc.vector.memset(hh[d][:], 0.0)
                    # software-pipelined: f-MMs(ss) | b-gates(ss-1) | b-MMs(ss) | f-gates(ss)
                    pend_b = None
                    for ss in range(S):
                        if "p2" in skip:
                            break
                        tf, tb = ss, S - 1 - ss
                        pg_f = dir_mms("f", tf, tf - 1 if ss else None)
                        if pend_b is not None:
                            dir_gates("b", pend_b[0], pend_b[1])
                        pg_b = dir_mms("b", tb, tb + 1 if ss else None)
                        dir_gates("f", tf, pg_f)
                        pend_b = (tb, pg_b)
                    if pend_b is not None:
                        dir_gates("b", pend_b[0], pend_b[1])

                # ---------------- P3: emissions ----------------
                with tc.tile_pool(name="p3ps", bufs=4, space="PSUM") as p3ps:
                  if "p3" not in skip:
                    for n in range(0, T, 512):
                        nn_ = min(512, T - n)
                        pe = p3ps.tile([12, 512], f32, tag="pe")
                        for c in range(6):
                            hsrc = hh["f"] if c < 3 else hh["b"]
                            cc = c % 3
                            nc.tensor.matmul(
                                out=pe[:, 0:nn_], lhsT=plin[:, 12 * c:12 * (c + 1)],
                                rhs=hsrc[:, T * cc + n:T * cc + n + nn_],
                                start=(c == 0), stop=(c == 5))
                        nc.vector.tensor_scalar(
                            out=emit[:, n:n + nn_], in0=pe[:, 0:nn_],
                            scalar1=blin[:, 0:1], scalar2=None, op0=mybir.AluOpType.add)

                # ---------------- P4: gold score ----------------
                with tc.tile_pool(name="p4", bufs=2) as p4:
                  if "p4" in skip:
                    nc.vector.memset(goldT[:], 0.0)
                  else:
                    s2 = p4.tile([12, T], f32, tag="s2")
                    with tc.tile_pool(name="p4psa", bufs=1, space="PSUM") as p4psa:
                        pts = p4psa.tile([12, T], f32, tag="pts")
                        for n in range(0, T, 512):
                            nn_ = min(512, T - n)
                            nc.tensor.matmul(out=pts[:, n:n + nn_], lhsT=transT_sb[:],
                                             rhs=mask[:, 8 + n:8 + n + nn_],
                                             start=True, stop=True)
                        nc.vector.tensor_add(out=s2[:], in0=pts[:], in1=emit[:])
                    nc.vector.tensor_mul(out=s2[:], in0=s2[:], in1=mask[:, 0:T])
                    p4ps_cm = tc.tile_pool(name="p4ps", bufs=1, space="PSUM")
                    p4ps = p4ps_cm.__enter__()
                    ps_s = p4ps.tile([1, T], f32, tag="ps_s")
                    for n in range(0, T, 512):
                        nn_ = min(512, T - n)
                        nc.tensor.matmul(out=ps_s[:, n:n + nn_], lhsT=ones12[:],
                                         rhs=s2[:, n:n + nn_], start=True, stop=True)
                    nc.vector.tensor_reduce(
                        out=goldT[:], in_=ps_s[:].rearrange("p (t b) -> p b t", b=8),
                        axis=mybir.AxisListType.X, op=mybir.AluOpType.add)
                    p4ps_cm.__exit__(None, None, None)

                # ---------------- P5: CRF alpha scan (p-space, 2 chains) ----------------
                # p_{t+1} = (Texp.T @ p_t) * exp(e_{t+1}); exp(emit) bulk-precomputed.
                # Batch split into two independent 4-wide chains to hide latency.
                nc.vector.memset(Mrow[:], 0.0)
                Ee = cp.tile([12, T], f32, name="Ee_sb")
                nc.scalar.activation(out=Ee[:], in_=emit[:],
                                     func=mybir.ActivationFunctionType.Exp)
                nc.vector.tensor_copy(out=D[:], in_=Ee[:, 0:8])
                with tc.tile_pool(name="p5", bufs=4) as p5, \
                     tc.tile_pool(name="p5ps", bufs=3, space="PSUM") as p5ps:
                    def refresh(h):
                        sl = slice(4 * h, 4 * h + 4)
                        pr = p5ps.tile([8, 12], f32, tag="scr", name=f"pr_{h}")
                        nc.tensor.transpose(out=pr[0:4, 0:12], in_=D[:, sl],
                                            identity=ident[0:12, 0:12])
                        m8 = p5.tile([4, 1], f32, tag=f"m8_{h}")
                        nc.vector.tensor_reduce(out=m8[:], in_=pr[0:4, 0:12],
                                                axis=mybir.AxisListType.X,
                                                op=mybir.AluOpType.max)
                        rm = p5.tile([4, 1], f32, tag=f"rm_{h}")
                        nc.vector.reciprocal(out=rm[:], in_=m8[:])
                        lnm = p5.tile([4, 1], f32, tag=f"lnm_{h}")
                        nc.scalar.activation(out=lnm[:], in_=m8[:],
                                             func=mybir.ActivationFunctionType.Ln,
                                             bias=eps_b[0:4, 0:1])
                        lnt = p5ps.tile([1, 4], f32, tag="scr", name=f"lnt_{h}")
                        nc.tensor.transpose(out=lnt[0:1, 0:4], in_=lnm[:],
                                            identity=ident[0:4, 0:4])
                        nc.vector.tensor_add(out=Mrow[:, sl], in0=Mrow[:, sl],
                                             in1=lnt[0:1, 0:4])
                        sh = p5.tile([4, 12], f32, tag=f"sh_{h}")
                        nc.vector.tensor_scalar(out=sh[:], in0=pr[0:4, 0:12],
                                                scalar1=rm[:, 0:1], scalar2=None,
                                                op0=mybir.AluOpType.mult)
                        pr2 = p5ps.tile([12, 4], f32, tag="scr", name=f"pr2_{h}")
                        nc.tensor.transpose(out=pr2[0:12, 0:4], in_=sh[:],
                                            identity=ident[0:4, 0:4])
                        nc.vector.tensor_copy(out=D[:, sl], in_=pr2[0:12, 0:4])

                    for t in range(1, S):
                        if "p5" in skip:
                            break
                        if t % 8 == 0:
                            refresh(0)
                            refresh(1)
                        pq0 = p5ps.tile([12, 4], f32, tag="pq0", bufs=2)
                        pq1 = p5ps.tile([12, 4], f32, tag="pq1", bufs=2)
                        nc.tensor.matmul(out=pq0[:], lhsT=texp[:], rhs=D[:, 0:4],
                                         start=True, stop=True)
                        nc.tensor.matmul(out=pq1[:], lhsT=texp[:], rhs=D[:, 4:8],
                                         start=True, stop=True)
                        nc.vector.tensor_mul(out=D[:, 0:4], in0=pq0[:],
                                             in1=Ee[:, 8 * t:8 * t + 4])
                        nc.vector.tensor_mul(out=D[:, 4:8], in0=pq1[:],
                                             in1=Ee[:, 8 * t + 4:8 * t + 8])

                    # ---------------- P6: finalize ----------------
                    pz = p5ps.tile([1, 8], f32, tag="scr", name="pz_f")
                    nc.tensor.matmul(out=pz[:], lhsT=ones12[:], rhs=D[:],
                                     start=True, stop=True)
                    zrow = p5.tile([1, 8], f32, tag="zrow")
                    nc.scalar.activation(out=zrow[:], in_=pz[:],
                                         func=mybir.ActivationFunctionType.Ln,
                                         bias=eps_b[0:1, 0:1])
                    nc.vector.tensor_add(out=zrow[:], in0=zrow[:], in1=Mrow[:])
                    nc.vector.tensor_scalar_add(out=zrow[:], in0=zrow[:],
                                                scalar1=float(3.0 * (S - 1)))
                    nc.vector.tensor_sub(out=zrow[:], in0=zrow[:], in1=goldT[:])
                    plt = p5ps.tile([8, 1], f32, tag="scr", name="plt_f")
                    nc.tensor.transpose(out=plt[0:8, 0:1], in_=zrow[:],
                                        identity=ident[0:1, 0:1])
                    nc.vector.tensor_copy(out=loss_sb[:], in_=plt[0:8, 0:1])
                nc.sync.dma_start(out=d_loss[:], in_=loss_sb[:])

    nc.compile()
    return nc, names


def _prepare_inputs(inputs, S):
    """Host-side packing: layout transforms only. Returns list of per-core maps."""
    from concourse import mybir
    fp8_np = mybir.dt.np(mybir.dt.float8e4)
    sent = np.asarray(inputs["sentences"]).astype(np.int32)
    tags = np.asarray(inputs["tags"]).astype(np.int32)
    embed = np.asarray(inputs["embed_table"], np.float32)
    packed = dict(
        pih_f=_pack_w(np.asarray(inputs["W_ih_f"])),
        phh_f=None,
        pih_b=_pack_w(np.asarray(inputs["W_ih_b"])),
        phh_b=None,
        bcol_f=_pack_b(np.asarray(inputs["b_f"])),
        bcol_b=_pack_b(np.asarray(inputs["b_b"])),
        plin=_pack_lin(np.asarray(inputs["W_lin"])),
        blin=np.ascontiguousarray(np.asarray(inputs["b_lin"], np.float32)[:, None]),
        trans=np.asarray(inputs["transitions"], np.float32),
        transT=np.ascontiguousarray(np.asarray(inputs["transitions"], np.float32).T),
        embed=embed,
    )
    packed["phh_f"] = _pack_w_fp8(np.asarray(inputs["W_hh_f"]), fp8_np)
    packed["phh_b"] = _pack_w_fp8(np.asarray(inputs["W_hh_b"]), fp8_np)
    maps = []
    for core in range(NCORES):
        sl = slice(core * BC, (core + 1) * BC)
        m = dict(packed)
        m["sent"] = np.ascontiguousarray(sent[sl, :S].T.reshape(-1))
        m["tags"] = np.ascontiguousarray(tags[sl, :S].T.reshape(-1))
        maps.append(m)
    return maps


def kernel(**inputs):
    from concourse import bass_utils
    S = 256
    if "k" + "ernel_S" in _cache:
        S = _cache["kernel_S"]
    if ("nc", S) not in _cache:
        _cache[("nc", S)] = build(S)
    nc, names = _cache[("nc", S)]
    maps = _prepare_inputs(inputs, S)
    in_maps = [{names[k]: v for k, v in m.items() if k != "loss"} for m in maps]
    res = bass_utils.run_bass_kernel_spmd(nc, in_maps, core_ids=list(range(NCORES)),
                                          trace=False)
    out = np.concatenate([r[names["loss"]].reshape(BC) for r in res.results])
    return out.astype(np.float32)


if __name__ == "__main__":
    import reference
    inputs = {k: np.asarray(v) for k, v in reference.setup_inputs().items()}
    expected = np.asarray(reference.reference(**inputs))
    actual = kernel(**inputs)
    rel = np.linalg.norm(actual - expected) / np.linalg.norm(expected)
    print("expected[:4]:", expected[:4])
    print("actual[:4]:  ", actual[:4])
    print("Relative error:", rel)



# revision 10
# speedup vs baseline: 1.5010x; 1.5010x over previous
"""BiLSTM-CRF NER loss kernel for 8 Trainium2 NeuronCores.

Strategy: data-parallel, 8 examples per core. Per core:
  P0  embedding gather (indirect DMA) + PE transpose -> xT [E-on-partitions]
      bf16, plus a constant ones-row at chunk-2 partition 44 that carries the
      gate bias through the x-projection matmuls.
  P2  fwd+bwd LSTM, two phase-shifted chains. Per direction and step-pair the
      x-projection (W_ih, scaled x16, g-gate x32) is matmul'd straight into
      the PSUM pair bank; per step the fp8 recurrence matmuls accumulate on
      top, one Sigmoid over all four gates (tanh via 2*sigmoid(2x)-1 with the
      x2 folded into the g-gate weight scale), then a short DVE cell chain.
      h is stored at half scale (W_hh and W_lin carry the compensating x2).
  P3  emission matmul -> emit [12 tags on partitions, 2048 tok] f32
  P4  gold path score via one-hot mask + transition-select matmul
  P5  CRF partition: pair-fused scan. K'_r = texp@diag(E_mid)@texp built in
      bulk (batched leaf STT + small matmuls), then 127 serial rounds of
      p <- diag(E)@K_r@p over two 4-example chains. No renormalization: a
      fixed exp(-3.5) shift per time step rides in texp and is restored as a
      constant at the end (drift stays well inside fp32 range).
  P6  loss = log_z - gold -> DRAM [8]
"""
import sys
sys.path.insert(0, '/opt/trn_rl_repo/concourse')
sys.path.insert(0, '/opt/trn_rl_repo')
import numpy as np
import ml_dtypes

E = 300
H = 300
NT = 12
BC = 8          # batch per core
NCORES = 8
DELTA = 3.5     # per-step log shift folded into texp

_cache = {}


def _bf16(x):
    return np.asarray(x).astype(ml_dtypes.bfloat16)


def _chunk(P):
    """(384, 1536) -> [128, 3*1536], K-chunk c at cols 1536c."""
    packed = np.zeros((128, 3 * 1536), np.float32)
    for c in range(3):
        packed[:, 1536 * c:1536 * (c + 1)] = P[128 * c:128 * (c + 1), :]
    return packed

# gate slot order i, f, o, g; scales: base x16, g-gate x32 (sigmoid-trick)
_SLOT_GATES = (0, 1, 3, 2)
_SLOT_SCALE = (16.0, 16.0, 16.0, 32.0)


def _pack_xw(W, b):
    """x-proj lhsT with bias on contraction row 300 (= chunk2 partition 44)."""
    P = np.zeros((384, 1536), np.float32)
    for slot, g in enumerate(_SLOT_GATES):
        sc = _SLOT_SCALE[slot]
        P[:300, 384 * slot:384 * slot + 300] = W[300 * g:300 * g + 300, :].T * sc
        P[300, 384 * slot:384 * slot + 300] = b[300 * g:300 * g + 300] * sc
    return _bf16(_chunk(P))


def _pack_hw(W, fp8_np):
    """recurrence lhsT fp8; x2 on top of x-proj scales compensates h/2."""
    P = np.zeros((384, 1536), np.float32)
    for slot, g in enumerate(_SLOT_GATES):
        sc = 2.0 * _SLOT_SCALE[slot]
        P[:300, 384 * slot:384 * slot + 300] = W[300 * g:300 * g + 300, :].T * sc
    return _chunk(P).astype(fp8_np)


def _pack_lin(W_lin):
    """emission lhsT [128, 6*12] bf16, x2 compensates h/2."""
    P = np.zeros((768, 12), np.float32)
    P[0:300, :] = W_lin[:, 0:300].T * 2.0
    P[384:684, :] = W_lin[:, 300:600].T * 2.0
    packed = np.zeros((128, 6 * 12), np.float32)
    for c in range(6):
        packed[:, 12 * c:12 * (c + 1)] = P[128 * c:128 * (c + 1), :]
    return _bf16(packed)


def build(S=256, skip=()):
    """Build + compile the bass program. Returns (nc, names)."""
    from concourse import bass, mybir, bacc
    import concourse.tile as tile
    from concourse.masks import make_identity

    T = S * BC
    NG = T // 128            # 128-token gather groups
    NR = (S - 1) // 2        # CRF pair rounds (127), +1 leftover single step
    f32 = mybir.dt.float32
    bf = mybir.dt.bfloat16
    i32 = mybir.dt.int32
    fp8 = mybir.dt.float8e4
    AF = mybir.ActivationFunctionType
    OP = mybir.AluOpType

    nc = bacc.Bacc("TRN2", target_bir_lowering=False, debug=False)
    names = {}
    with tile.TileContext(nc) as tc:
        with tc.tile_pool(name="dram", bufs=1, space="DRAM") as dram:
            d_sent = dram.tile([T], i32, kind="ExternalInput", name="sent")
            d_tags = dram.tile([T], i32, kind="ExternalInput", name="tags")
            d_embed = dram.tile([50000, E], f32, kind="ExternalInput", name="embed")
            d_pxw_f = dram.tile([128, 4608], bf, kind="ExternalInput", name="pxw_f")
            d_pxw_b = dram.tile([128, 4608], bf, kind="ExternalInput", name="pxw_b")
            d_phw_f = dram.tile([128, 4608], fp8, kind="ExternalInput", name="phw_f")
            d_phw_b = dram.tile([128, 4608], fp8, kind="ExternalInput", name="phw_b")
            d_plin = dram.tile([128, 72], bf, kind="ExternalInput", name="plin")
            d_blin = dram.tile([12, 1], f32, kind="ExternalInput", name="blin")
            d_transT = dram.tile([12, 12], f32, kind="ExternalInput", name="transT")
            d_texp = dram.tile([12, 12], bf, kind="ExternalInput", name="texp")
            d_texpT8 = dram.tile([12, 96], bf, kind="ExternalInput", name="texpT8")
            d_loss = dram.tile([8, 1], f32, kind="ExternalOutput", name="loss")
            for k, v in [("sent", d_sent), ("tags", d_tags), ("embed", d_embed),
                         ("pxw_f", d_pxw_f), ("pxw_b", d_pxw_b),
                         ("phw_f", d_phw_f), ("phw_b", d_phw_b),
                         ("plin", d_plin), ("blin", d_blin), ("transT", d_transT),
                         ("texp", d_texp), ("texpT8", d_texpT8), ("loss", d_loss)]:
                names[k] = v.name

            with tc.tile_pool(name="const", bufs=1) as cp:
                ident = cp.tile([128, 128], f32)
                make_identity(nc, ident[:])
                pxw = {"f": cp.tile([128, 4608], bf, name="pxw_f_sb"),
                       "b": cp.tile([128, 4608], bf, name="pxw_b_sb")}
                phw = {"f": cp.tile([128, 4608], fp8, name="phw_f_sb"),
                       "b": cp.tile([128, 4608], fp8, name="phw_b_sb")}
                plin = cp.tile([128, 72], bf)
                blin = cp.tile([12, 1], f32)
                transT_sb = cp.tile([12, 12], f32)
                texp_sb = cp.tile([12, 12], bf)
                texpT8_sb = cp.tile([12, 96], bf)
                ones12 = cp.tile([12, 1], bf)
                iota_f = cp.tile([12, 1], f32)
                eps_b = cp.tile([12, 1], f32)
                nc.vector.memset(eps_b[:], 1e-30)
                nc.vector.memset(ones12[:], 1.0)
                for d in "fb":
                    nc.sync.dma_start(out=pxw[d][:], in_=(d_pxw_f if d == "f" else d_pxw_b)[:])
                    nc.sync.dma_start(out=phw[d][:], in_=(d_phw_f if d == "f" else d_phw_b)[:])
                nc.sync.dma_start(out=plin[:], in_=d_plin[:])
                nc.sync.dma_start(out=blin[:], in_=d_blin[:])
                nc.sync.dma_start(out=transT_sb[:], in_=d_transT[:])
                nc.sync.dma_start(out=texp_sb[:], in_=d_texp[:])
                nc.sync.dma_start(out=texpT8_sb[:], in_=d_texpT8[:])
                with tc.tile_pool(name="iota_tmp", bufs=1) as itp:
                    iota_i = itp.tile([12, 1], i32)
                    nc.gpsimd.iota(out=iota_i[:], pattern=[[0, 1]], base=0,
                                   channel_multiplier=1)
                    nc.vector.tensor_copy(out=iota_f[:], in_=iota_i[:])

                # persistent tensors
                xT = cp.tile([128, 3 * T], bf, name="xT_sb")
                hh = {"f": cp.tile([128, 3 * T], bf, name="hh_f_sb"),
                      "b": cp.tile([128, 3 * T], bf, name="hh_b_sb")}
                emit = cp.tile([12, T], f32)
                Ee = cp.tile([12, T], bf, name="Ee_sb")
                mask = cp.tile([12, T + 8], f32)
                goldT = cp.tile([1, 8], f32)
                D = cp.tile([12, 8], bf)
                loss_sb = cp.tile([8, 1], f32)

                # ---------------- P0: gather + transpose ----------------
                nc.vector.memset(xT[:, 2 * T:3 * T], 0.0)
                # bias ones-row at partition 44 (32-aligned block; rows 32-43
                # are overwritten by the transposes, rows 45-63 hit zero weights)
                nc.vector.memset(xT[32:64, 2 * T:3 * T], 1.0)
                with tc.tile_pool(name="p0", bufs=4) as p0, \
                     tc.tile_pool(name="p0ps", bufs=4, space="PSUM") as p0ps:
                  if "p0" not in skip:
                    idx = p0.tile([128, NG], i32, tag="idx")
                    nc.sync.dma_start(
                        out=idx[:], in_=d_sent[:].rearrange("(g p) -> p g", p=128))
                    # ends-first order so both LSTM chains can start early
                    g_order = [g for pair in zip(range(NG - 1, -1, -1), range(NG))
                               for g in pair][:NG]
                    for g in g_order:
                        xr = p0.tile([128, E], f32, tag="xr")
                        nc.gpsimd.indirect_dma_start(
                            out=xr[:], out_offset=None, in_=d_embed[:],
                            in_offset=bass.IndirectOffsetOnAxis(ap=idx[:, g:g + 1], axis=0))
                        for s, (lo, sz) in enumerate([(0, 128), (128, 128), (256, 44)]):
                            pt = p0ps.tile([128, 128], f32, tag="pt")
                            nc.tensor.transpose(out=pt[0:sz, :], in_=xr[:, lo:lo + sz],
                                                identity=ident[:])
                            nc.vector.tensor_copy(
                                out=xT[0:sz, T * s + 128 * g: T * s + 128 * (g + 1)],
                                in_=pt[0:sz, :])

                # tags broadcast to 12 partitions + mask build
                with tc.tile_pool(name="ptg", bufs=1) as ptg:
                  if "ptg" not in skip:
                    tagsr = ptg.tile([12, T], i32, tag="tagsr")
                    for j in range(12):
                        nc.sync.dma_start(out=tagsr[j:j + 1, :],
                                          in_=d_tags[:].rearrange("(a t) -> a t", a=1))
                    tags_f = ptg.tile([12, T], f32, tag="tagsf")
                    nc.vector.tensor_copy(out=tags_f[:], in_=tagsr[:])
                    nc.vector.memset(mask[:, T:T + 8], 0.0)
                    nc.vector.tensor_scalar(
                        out=mask[:, 0:T], in0=tags_f[:], scalar1=iota_f[:, 0:1],
                        scalar2=None, op0=OP.is_equal)

                # ---------------- P2: recurrences, xpair structure ----------
                with tc.tile_pool(name="p2", bufs=4) as p2, \
                     tc.tile_pool(name="p2c", bufs=1) as p2c, \
                     tc.tile_pool(name="p2ps", bufs=4, space="PSUM") as p2ps:
                    cst = {d: p2c.tile([128, 24], f32, tag=f"c_{d}", name=f"cst_{d}")
                           for d in "fb"}
                    for d in "fb":
                        nc.vector.memset(cst[d][:], 0.0)

                    pair_tiles = {}

                    def xpair(d, pr):
                        """psum [128, 192]: x-proj (+bias) for a step pair."""
                        key = (d, pr)
                        if key in pair_tiles:
                            return pair_tiles[key]
                        pg = p2ps.tile([128, 192], f32, tag=f"pp_{d}",
                                       name=f"pp_{d}_{pr}", bufs=3)
                        t_lo = 2 * pr if d == "f" else S - 2 - 2 * pr
                        for m in range(12):
                            for c in range(3):
                                nc.tensor.matmul(
                                    out=pg[:].rearrange("q (s x) -> q s x", s=2)[:, :, 8 * m:8 * (m + 1)],
                                    lhsT=pxw[d][:, 1536 * c + 128 * m:1536 * c + 128 * (m + 1)],
                                    rhs=xT[:, T * c + 8 * t_lo:T * c + 8 * t_lo + 16],
                                    start=(c == 0), stop=(c == 2),
                                    skip_group_check=True)
                        pair_tiles[key] = pg
                        return pg

                    def step_mms(d, t):
                        # pair index / half within the pair psum
                        if d == "f":
                            pr, half, tprev = t // 2, t % 2, t - 1
                        else:
                            k = S - 1 - t
                            pr, half, tprev = k // 2, 1 - (k % 2), t + 1
                        pg = xpair(d, pr)
                        ph = pg[:, 96 * half:96 * half + 96]
                        first = (t == 0) if d == "f" else (t == S - 1)
                        if not first and "norecur" not in skip:
                            for m in range(12):
                                for c in range(3):
                                    nc.tensor.matmul(
                                        out=ph[:, 8 * m:8 * (m + 1)],
                                        lhsT=phw[d][:, 1536 * c + 128 * m:1536 * c + 128 * (m + 1)],
                                        rhs=hh[d][:, T * c + 8 * tprev:T * c + 8 * tprev + 8],
                                        start=False, stop=(c == 2),
                                        skip_group_check=True)
                        return ph

                    def step_tail(d, t, ph):
                        gact = p2.tile([128, 96], bf, tag=f"g_{d}", name=f"g_{d}_{t}")
                        nc.scalar.activation(out=gact[:], in_=ph,
                                             func=AF.Sigmoid, scale=0.0625)
                        # i:0-24 f:24-48 o:48-72 g':72-96
                        ig = p2.tile([128, 24], bf, tag=f"ig_{d}")
                        nc.vector.scalar_tensor_tensor(
                            out=ig[:], in0=gact[:, 72:96], scalar=-0.5,
                            in1=gact[:, 0:24], op0=OP.subtract, op1=OP.mult)
                        fc = p2.tile([128, 24], f32, tag=f"fc_{d}")
                        nc.vector.tensor_mul(out=fc[:], in0=gact[:, 24:48],
                                             in1=cst[d][:])
                        nc.vector.scalar_tensor_tensor(
                            out=cst[d][:], in0=ig[:], scalar=2.0, in1=fc[:],
                            op0=OP.mult, op1=OP.add)
                        tc_t = p2.tile([128, 24], bf, tag=f"tc_{d}")
                        nc.scalar.activation(out=tc_t[:], in_=cst[d][:],
                                             func=AF.Sigmoid, scale=2.0)
                        # h/2 = (sig(2c)-0.5)*o  -> hh (3-chunk layout)
                        nc.vector.scalar_tensor_tensor(
                            out=hh[d][:].rearrange("p (c x) -> p c x", c=3)[:, :, 8 * t:8 * t + 8],
                            in0=tc_t[:].rearrange("p (c x) -> p c x", c=3),
                            scalar=-0.5,
                            in1=gact[:, 48:72].rearrange("p (c x) -> p c x", c=3),
                            op0=OP.subtract, op1=OP.mult)

                    if "p2" in skip:
                        for d in "fb":
                            nc.vector.memset(hh[d][:], 0.0)
                    else:
                        # skewed pipeline: f-MMs(ss) | b-tail(ss-1) | b-MMs(ss) | f-tail(ss)
                        pend_b = None
                        for ss in range(S):
                            tf, tb = ss, S - 1 - ss
                            ph_f = step_mms("f", tf)
                            if pend_b is not None:
                                step_tail("b", pend_b[0], pend_b[1])
                            ph_b = step_mms("b", tb)
                            step_tail("f", tf, ph_f)
                            pend_b = (tb, ph_b)
                        step_tail("b", pend_b[0], pend_b[1])

                # ---------------- P3: emissions ----------------
                with tc.tile_pool(name="p3ps", bufs=4, space="PSUM") as p3ps:
                  if "p3" not in skip:
                    for n in range(0, T, 512):
                        pe = p3ps.tile([12, 512], f32, tag="pe")
                        for c in range(6):
                            hsrc = hh["f"] if c < 3 else hh["b"]
                            cc = c % 3
                            nc.tensor.matmul(
                                out=pe[:, 0:512], lhsT=plin[:, 12 * c:12 * (c + 1)],
                                rhs=hsrc[:, T * cc + n:T * cc + n + 512],
                                start=(c == 0), stop=(c == 5))
                        nc.vector.tensor_scalar(
                            out=emit[:, n:n + 512], in0=pe[:, 0:512],
                            scalar1=blin[:, 0:1], scalar2=None, op0=OP.add)
                    nc.scalar.activation(out=Ee[:], in_=emit[:], func=AF.Exp)

                # ---------------- P4: gold score ----------------
                with tc.tile_pool(name="p4", bufs=2) as p4:
                  if "p4" in skip:
                    nc.vector.memset(goldT[:], 0.0)
                  else:
                    s2 = p4.tile([12, T], f32, tag="s2")
                    with tc.tile_pool(name="p4psa", bufs=1, space="PSUM") as p4psa:
                        pts = p4psa.tile([12, T], f32, tag="pts")
                        for n in range(0, T, 512):
                            nc.tensor.matmul(out=pts[:, n:n + 512], lhsT=transT_sb[:],
                                             rhs=mask[:, 8 + n:8 + n + 512],
                                             start=True, stop=True)
                        nc.vector.tensor_add(out=s2[:], in0=pts[:], in1=emit[:])
                    nc.vector.tensor_mul(out=s2[:], in0=s2[:], in1=mask[:, 0:T])
                    with tc.tile_pool(name="p4ps", bufs=1, space="PSUM") as p4ps:
                        ps_s = p4ps.tile([1, T], f32, tag="ps_s")
                        ones12f = p4.tile([12, 1], f32, tag="onesf")
                        nc.vector.memset(ones12f[:], 1.0)
                        for n in range(0, T, 512):
                            nc.tensor.matmul(out=ps_s[:, n:n + 512], lhsT=ones12f[:],
                                             rhs=s2[:, n:n + 512], start=True, stop=True)
                        nc.vector.tensor_reduce(
                            out=goldT[:], in_=ps_s[:].rearrange("p (t b) -> p b t", b=8),
                            axis=mybir.AxisListType.X, op=OP.add)

                # ---------------- P5: pair-fused CRF scan ----------------
                nc.vector.tensor_copy(out=D[:], in_=Ee[:, 0:8])
                with tc.tile_pool(name="p5k", bufs=1) as p5k, \
                     tc.tile_pool(name="p5", bufs=4) as p5, \
                     tc.tile_pool(name="p5ps", bufs=1, space="PSUM") as p5ps:
                  if "p5" not in skip:
                    # leaves: leafT[r, j] = diag(Ee[:, 8(2r+1)+j]) @ texpT
                    leafT = p5k.tile([12, NR * 96], bf, name="leafT_sb")
                    for j in range(BC):
                        src = Ee[:, 8 + j:8 + j + NR * 16]
                        src3 = src.rearrange("p (r x) -> p r x", x=16)[:, :, 0:1]
                        nc.vector.scalar_tensor_tensor(
                            out=leafT[:].rearrange("p (r e k) -> p r e k", e=BC, k=12)[:, :, j, :],
                            in0=src3.broadcast_to([12, NR, 12]),
                            scalar=1.0, in1=texpT8_sb[:, 12 * j:12 * j + 12]
                                 .unsqueeze(1).broadcast_to([12, NR, 12]),
                            op0=OP.mult, op1=OP.mult)
                    # K'_r(j) = leafT(r,j).T @ texp  (batches of 5 rounds/bank)
                    Ksb = p5k.tile([12, NR * 96], bf, name="Ksb")
                    for r0 in range(0, NR, 5):
                        nb = min(5, NR - r0)
                        pk = p5ps.tile([12, 480], f32, tag="pk", bufs=2)
                        for r in range(r0, r0 + nb):
                            for j in range(BC):
                                nc.tensor.matmul(
                                    out=pk[:, 96 * (r - r0) + 12 * j:96 * (r - r0) + 12 * j + 12],
                                    lhsT=leafT[:, 96 * r + 12 * j:96 * r + 12 * j + 12],
                                    rhs=texp_sb[:], start=True, stop=True)
                        nc.vector.tensor_copy(out=Ksb[:, 96 * r0:96 * r0 + 96 * nb],
                                              in_=pk[:, 0:96 * nb])
                    # serial rounds: two 4-example chains
                    for r in range(NR):
                        for h in range(2):
                            pq = p5ps.tile([12, 4], f32, tag=f"pq{h}", bufs=2)
                            for j in range(4):
                                ex = 4 * h + j
                                nc.tensor.matmul(
                                    out=pq[:, j:j + 1],
                                    lhsT=Ksb[:, 96 * r + 12 * ex:96 * r + 12 * ex + 12],
                                    rhs=D[:, ex:ex + 1], start=True, stop=True)
                            nc.vector.tensor_mul(
                                out=D[:, 4 * h:4 * h + 4], in0=pq[:],
                                in1=Ee[:, 8 * (2 * r + 2) + 4 * h:8 * (2 * r + 2) + 4 * h + 4])
                    # leftover single step t = S-1 (odd tail)
                    pq0 = p5ps.tile([12, 4], f32, tag="pq0", bufs=2)
                    pq1 = p5ps.tile([12, 4], f32, tag="pq1", bufs=2)
                    nc.tensor.matmul(out=pq0[:], lhsT=texp_sb[:], rhs=D[:, 0:4],
                                     start=True, stop=True)
                    nc.tensor.matmul(out=pq1[:], lhsT=texp_sb[:], rhs=D[:, 4:8],
                                     start=True, stop=True)
                    nc.vector.tensor_mul(out=D[:, 0:4], in0=pq0[:],
                                         in1=Ee[:, 8 * (S - 1):8 * (S - 1) + 4])
                    nc.vector.tensor_mul(out=D[:, 4:8], in0=pq1[:],
                                         in1=Ee[:, 8 * (S - 1) + 4:8 * S])

                    # ---------------- P6: finalize ----------------
                    pz = p5ps.tile([1, 8], f32, tag="pz", name="pz_f")
                    nc.tensor.matmul(out=pz[:], lhsT=ones12[:], rhs=D[:],
                                     start=True, stop=True)
                    zrow = p5.tile([1, 8], f32, tag="zrow")
                    nc.scalar.activation(out=zrow[:], in_=pz[:], func=AF.Ln,
                                         bias=eps_b[0:1, 0:1])
                    nc.vector.tensor_scalar_add(out=zrow[:], in0=zrow[:],
                                                scalar1=float(DELTA * (S - 1)))
                    nc.vector.tensor_sub(out=zrow[:], in0=zrow[:], in1=goldT[:])
                    plt = p5ps.tile([8, 1], f32, tag="plt", name="plt_f")
                    nc.tensor.transpose(out=plt[0:8, 0:1], in_=zrow[:],
                                        identity=ident[0:1, 0:1])
                    nc.vector.tensor_copy(out=loss_sb[:], in_=plt[0:8, 0:1])
                  else:
                    nc.vector.memset(loss_sb[:], 0.0)
                nc.sync.dma_start(out=d_loss[:], in_=loss_sb[:])

    nc.compile()
    return nc, names


def _prepare_inputs(inputs, S):
    """Host-side packing: layout transforms only. Returns list of per-core maps."""
    from concourse import mybir
    fp8_np = mybir.dt.np(mybir.dt.float8e4)
    sent = np.asarray(inputs["sentences"]).astype(np.int32)
    tags = np.asarray(inputs["tags"]).astype(np.int32)
    trans = np.asarray(inputs["transitions"], np.float32)
    texp = np.exp(trans - DELTA)
    packed = dict(
        pxw_f=_pack_xw(np.asarray(inputs["W_ih_f"]), np.asarray(inputs["b_f"])),
        pxw_b=_pack_xw(np.asarray(inputs["W_ih_b"]), np.asarray(inputs["b_b"])),
        phw_f=_pack_hw(np.asarray(inputs["W_hh_f"]), fp8_np),
        phw_b=_pack_hw(np.asarray(inputs["W_hh_b"]), fp8_np),
        plin=_pack_lin(np.asarray(inputs["W_lin"])),
        blin=np.ascontiguousarray(np.asarray(inputs["b_lin"], np.float32)[:, None]),
        transT=np.ascontiguousarray(trans.T),
        texp=_bf16(texp),
        texpT8=_bf16(np.tile(np.ascontiguousarray(texp.T), (1, 8))),
        embed=np.asarray(inputs["embed_table"], np.float32),
    )
    maps = []
    for core in range(NCORES):
        sl = slice(core * BC, (core + 1) * BC)
        m = dict(packed)
        m["sent"] = np.ascontiguousarray(sent[sl, :S].T.reshape(-1))
        m["tags"] = np.ascontiguousarray(tags[sl, :S].T.reshape(-1))
        maps.append(m)
    return maps


def kernel(**inputs):
    from concourse import bass_utils
    S = 256
    if ("nc", S) not in _cache:
        _cache[("nc", S)] = build(S)
    nc, names = _cache[("nc", S)]
    maps = _prepare_inputs(inputs, S)
    in_maps = [{names[k]: v for k, v in m.items() if k != "loss"} for m in maps]
    res = bass_utils.run_bass_kernel_spmd(nc, in_maps, core_ids=list(range(NCORES)),
                                          trace=False)
    out = np.concatenate([r[names["loss"]].reshape(BC) for r in res.results])
    return out.astype(np.float32)


if __name__ == "__main__":
    import reference
    inputs = {k: np.asarray(v) for k, v in reference.setup_inputs().items()}
    expected = np.asarray(reference.reference(**inputs))
    actual = kernel(**inputs)
    rel = np.linalg.norm(actual - expected) / np.linalg.norm(expected)
    print("expected[:4]:", expected[:4])
    print("actual[:4]:  ", actual[:4])
    print("Relative error:", rel)


# revision 11
# speedup vs baseline: 1.5303x; 1.0195x over previous
"""BiLSTM-CRF NER loss kernel for 8 Trainium2 NeuronCores.

Strategy: data-parallel, 8 examples per core. Per core:
  P0  embedding gather (indirect DMA) + PE transpose -> xT [E-on-partitions]
      bf16, plus a constant ones-row at chunk-2 partition 44 that carries the
      gate bias through the x-projection matmuls.
  P2  fwd+bwd LSTM, two phase-shifted chains. Per direction and step-pair the
      x-projection (W_ih, scaled x16, g-gate x32) is matmul'd straight into
      the PSUM pair bank; per step the fp8 recurrence matmuls accumulate on
      top, one Sigmoid over all four gates (tanh via 2*sigmoid(2x)-1 with the
      x2 folded into the g-gate weight scale), then a short DVE cell chain.
      h is stored at half scale (W_hh and W_lin carry the compensating x2).
  P3  emission matmul -> emit [12 tags on partitions, 2048 tok] f32
  P4  gold path score via one-hot mask + transition-select matmul
  P5  CRF partition: pair-fused scan. K'_r = texp@diag(E_mid)@texp built in
      bulk (batched leaf STT + small matmuls), then 127 serial rounds of
      p <- diag(E)@K_r@p over two 4-example chains. No renormalization: a
      fixed exp(-3.5) shift per time step rides in texp and is restored as a
      constant at the end (drift stays well inside fp32 range).
  P6  loss = log_z - gold -> DRAM [8]
"""
import sys
sys.path.insert(0, '/opt/trn_rl_repo/concourse')
sys.path.insert(0, '/opt/trn_rl_repo')
import numpy as np
import ml_dtypes

E = 300
H = 300
NT = 12
BC = 8          # batch per core
NCORES = 8
DELTA = 3.5     # per-step log shift folded into texp

_cache = {}


def _bf16(x):
    return np.asarray(x).astype(ml_dtypes.bfloat16)


def _chunk(P):
    """(384, 1536) -> [128, 3*1536], K-chunk c at cols 1536c."""
    packed = np.zeros((128, 3 * 1536), np.float32)
    for c in range(3):
        packed[:, 1536 * c:1536 * (c + 1)] = P[128 * c:128 * (c + 1), :]
    return packed

# gate slot order i, f, o, g; scales: base x16, g-gate x32 (sigmoid-trick)
_SLOT_GATES = (0, 1, 3, 2)
_SLOT_SCALE = (16.0, 16.0, 16.0, 32.0)


def _pack_xw(W, b):
    """x-proj lhsT with bias on contraction row 300 (= chunk2 partition 44)."""
    P = np.zeros((384, 1536), np.float32)
    for slot, g in enumerate(_SLOT_GATES):
        sc = _SLOT_SCALE[slot]
        P[:300, 384 * slot:384 * slot + 300] = W[300 * g:300 * g + 300, :].T * sc
        P[300, 384 * slot:384 * slot + 300] = b[300 * g:300 * g + 300] * sc
    return _bf16(_chunk(P))


def _pack_hw(W, fp8_np):
    """recurrence lhsT fp8; x2 on top of x-proj scales compensates h/2."""
    P = np.zeros((384, 1536), np.float32)
    for slot, g in enumerate(_SLOT_GATES):
        sc = 2.0 * _SLOT_SCALE[slot]
        P[:300, 384 * slot:384 * slot + 300] = W[300 * g:300 * g + 300, :].T * sc
    return _chunk(P).astype(fp8_np)


def _pack_lin(W_lin):
    """emission lhsT [128, 6*12] bf16, x2 compensates h/2."""
    P = np.zeros((768, 12), np.float32)
    P[0:300, :] = W_lin[:, 0:300].T * 2.0
    P[384:684, :] = W_lin[:, 300:600].T * 2.0
    packed = np.zeros((128, 6 * 12), np.float32)
    for c in range(6):
        packed[:, 12 * c:12 * (c + 1)] = P[128 * c:128 * (c + 1), :]
    return _bf16(packed)


def build(S=256, skip=()):
    """Build + compile the bass program. Returns (nc, names)."""
    from concourse import bass, mybir, bacc
    import concourse.tile as tile
    from concourse.masks import make_identity

    T = S * BC
    NG = T // 128            # 128-token gather groups
    NR = (S - 1) // 2        # CRF pair rounds (127), +1 leftover single step
    f32 = mybir.dt.float32
    bf = mybir.dt.bfloat16
    i32 = mybir.dt.int32
    fp8 = mybir.dt.float8e4
    AF = mybir.ActivationFunctionType
    OP = mybir.AluOpType

    nc = bacc.Bacc("TRN2", target_bir_lowering=False, debug=False)
    names = {}
    with tile.TileContext(nc) as tc:
        with tc.tile_pool(name="dram", bufs=1, space="DRAM") as dram:
            d_sent = dram.tile([T], i32, kind="ExternalInput", name="sent")
            d_tags = dram.tile([T], i32, kind="ExternalInput", name="tags")
            d_embed = dram.tile([50000, E], f32, kind="ExternalInput", name="embed")
            d_pxw_f = dram.tile([128, 4608], bf, kind="ExternalInput", name="pxw_f")
            d_pxw_b = dram.tile([128, 4608], bf, kind="ExternalInput", name="pxw_b")
            d_phw_f = dram.tile([128, 4608], fp8, kind="ExternalInput", name="phw_f")
            d_phw_b = dram.tile([128, 4608], fp8, kind="ExternalInput", name="phw_b")
            d_plin = dram.tile([128, 72], bf, kind="ExternalInput", name="plin")
            d_blin = dram.tile([12, 1], f32, kind="ExternalInput", name="blin")
            d_transT = dram.tile([12, 12], bf, kind="ExternalInput", name="transT")
            d_texp = dram.tile([12, 12], bf, kind="ExternalInput", name="texp")
            d_texpT8 = dram.tile([12, 96], bf, kind="ExternalInput", name="texpT8")
            d_loss = dram.tile([8, 1], f32, kind="ExternalOutput", name="loss")
            for k, v in [("sent", d_sent), ("tags", d_tags), ("embed", d_embed),
                         ("pxw_f", d_pxw_f), ("pxw_b", d_pxw_b),
                         ("phw_f", d_phw_f), ("phw_b", d_phw_b),
                         ("plin", d_plin), ("blin", d_blin), ("transT", d_transT),
                         ("texp", d_texp), ("texpT8", d_texpT8), ("loss", d_loss)]:
                names[k] = v.name

            with tc.tile_pool(name="const", bufs=1) as cp:
                ident = cp.tile([128, 128], f32)
                make_identity(nc, ident[:])
                pxw = {"f": cp.tile([128, 4608], bf, name="pxw_f_sb"),
                       "b": cp.tile([128, 4608], bf, name="pxw_b_sb")}
                phw = {"f": cp.tile([128, 4608], fp8, name="phw_f_sb"),
                       "b": cp.tile([128, 4608], fp8, name="phw_b_sb")}
                plin = cp.tile([128, 72], bf)
                blin = cp.tile([12, 1], f32)
                transT_sb = cp.tile([12, 12], bf)
                texp_sb = cp.tile([12, 12], bf)
                texpT8_sb = cp.tile([12, 96], bf)
                ones12 = cp.tile([12, 1], bf)
                iota_f = cp.tile([12, 1], f32)
                eps_b = cp.tile([12, 1], f32)
                nc.vector.memset(eps_b[:], 1e-30)
                nc.vector.memset(ones12[:], 1.0)
                for d in "fb":
                    nc.sync.dma_start(out=pxw[d][:], in_=(d_pxw_f if d == "f" else d_pxw_b)[:])
                    nc.sync.dma_start(out=phw[d][:], in_=(d_phw_f if d == "f" else d_phw_b)[:])
                nc.sync.dma_start(out=plin[:], in_=d_plin[:])
                nc.sync.dma_start(out=blin[:], in_=d_blin[:])
                nc.sync.dma_start(out=transT_sb[:], in_=d_transT[:])
                nc.sync.dma_start(out=texp_sb[:], in_=d_texp[:])
                nc.sync.dma_start(out=texpT8_sb[:], in_=d_texpT8[:])
                with tc.tile_pool(name="iota_tmp", bufs=1) as itp:
                    iota_i = itp.tile([12, 1], i32)
                    nc.gpsimd.iota(out=iota_i[:], pattern=[[0, 1]], base=0,
                                   channel_multiplier=1)
                    nc.vector.tensor_copy(out=iota_f[:], in_=iota_i[:])

                # persistent tensors
                xT = cp.tile([128, 3 * T], bf, name="xT_sb")
                hh = {"f": cp.tile([128, 3 * T], bf, name="hh_f_sb"),
                      "b": cp.tile([128, 3 * T], bf, name="hh_b_sb")}
                emit = cp.tile([12, T], f32)
                Ee = cp.tile([12, T], bf, name="Ee_sb")
                mask = cp.tile([12, T + 8], bf)
                goldT = cp.tile([1, 8], f32)
                D = cp.tile([12, 8], bf)
                loss_sb = cp.tile([8, 1], f32)

                # ---------------- P0: gather + transpose ----------------
                nc.vector.memset(xT[:, 2 * T:3 * T], 0.0)
                # bias ones-row at partition 44 (32-aligned block; rows 32-43
                # are overwritten by the transposes, rows 45-63 hit zero weights)
                nc.vector.memset(xT[32:64, 2 * T:3 * T], 1.0)
                with tc.tile_pool(name="p0", bufs=4) as p0, \
                     tc.tile_pool(name="p0ps", bufs=4, space="PSUM") as p0ps:
                  if "p0" not in skip:
                    idx = p0.tile([128, NG], i32, tag="idx")
                    nc.sync.dma_start(
                        out=idx[:], in_=d_sent[:].rearrange("(g p) -> p g", p=128))
                    # ends-first order so both LSTM chains can start early
                    g_order = [g for pair in zip(range(NG - 1, -1, -1), range(NG))
                               for g in pair][:NG]
                    for g in g_order:
                        xr = p0.tile([128, E], f32, tag="xr")
                        nc.gpsimd.indirect_dma_start(
                            out=xr[:], out_offset=None, in_=d_embed[:],
                            in_offset=bass.IndirectOffsetOnAxis(ap=idx[:, g:g + 1], axis=0))
                        for s, (lo, sz) in enumerate([(0, 128), (128, 128), (256, 44)]):
                            pt = p0ps.tile([128, 128], f32, tag="pt")
                            nc.tensor.transpose(out=pt[0:sz, :], in_=xr[:, lo:lo + sz],
                                                identity=ident[:])
                            nc.vector.tensor_copy(
                                out=xT[0:sz, T * s + 128 * g: T * s + 128 * (g + 1)],
                                in_=pt[0:sz, :])

                # tags broadcast to 12 partitions + mask build
                with tc.tile_pool(name="ptg", bufs=1) as ptg:
                  if "ptg" not in skip:
                    tagsr = ptg.tile([12, T], i32, tag="tagsr")
                    for j in range(12):
                        nc.sync.dma_start(out=tagsr[j:j + 1, :],
                                          in_=d_tags[:].rearrange("(a t) -> a t", a=1))
                    tags_f = ptg.tile([12, T], f32, tag="tagsf")
                    nc.vector.tensor_copy(out=tags_f[:], in_=tagsr[:])
                    nc.vector.memset(mask[:, T:T + 8], 0.0)
                    nc.vector.tensor_scalar(
                        out=mask[:, 0:T], in0=tags_f[:], scalar1=iota_f[:, 0:1],
                        scalar2=None, op0=OP.is_equal)

                # ---------------- P2: recurrences, xpair structure ----------
                with tc.tile_pool(name="p2", bufs=4) as p2, \
                     tc.tile_pool(name="p2c", bufs=1) as p2c, \
                     tc.tile_pool(name="p2ps", bufs=4, space="PSUM") as p2ps:
                    cst = {d: p2c.tile([128, 24], bf, tag=f"c_{d}", name=f"cst_{d}")
                           for d in "fb"}
                    for d in "fb":
                        nc.vector.memset(cst[d][:], 0.0)

                    pair_tiles = {}

                    def xpair(d, pr):
                        """psum [128, 192]: x-proj (+bias) for a step pair."""
                        key = (d, pr)
                        if key in pair_tiles:
                            return pair_tiles[key]
                        pg = p2ps.tile([128, 192], f32, tag=f"pp_{d}",
                                       name=f"pp_{d}_{pr}", bufs=3)
                        t_lo = 2 * pr if d == "f" else S - 2 - 2 * pr
                        for m in range(12):
                            for c in range(3):
                                nc.tensor.matmul(
                                    out=pg[:].rearrange("q (s x) -> q s x", s=2)[:, :, 8 * m:8 * (m + 1)],
                                    lhsT=pxw[d][:, 1536 * c + 128 * m:1536 * c + 128 * (m + 1)],
                                    rhs=xT[:, T * c + 8 * t_lo:T * c + 8 * t_lo + 16],
                                    start=(c == 0), stop=(c == 2),
                                    skip_group_check=True)
                        pair_tiles[key] = pg
                        return pg

                    def step_mms(d, t):
                        # pair index / half within the pair psum
                        if d == "f":
                            pr, half, tprev = t // 2, t % 2, t - 1
                        else:
                            k = S - 1 - t
                            pr, half, tprev = k // 2, 1 - (k % 2), t + 1
                        pg = xpair(d, pr)
                        ph = pg[:, 96 * half:96 * half + 96]
                        first = (t == 0) if d == "f" else (t == S - 1)
                        if not first and "norecur" not in skip:
                            for m in range(12):
                                for c in range(3):
                                    nc.tensor.matmul(
                                        out=ph[:, 8 * m:8 * (m + 1)],
                                        lhsT=phw[d][:, 1536 * c + 128 * m:1536 * c + 128 * (m + 1)],
                                        rhs=hh[d][:, T * c + 8 * tprev:T * c + 8 * tprev + 8],
                                        start=False, stop=(c == 2),
                                        skip_group_check=True)
                        return ph

                    def step_tail(d, t, ph):
                        gact = p2.tile([128, 96], bf, tag=f"g_{d}", name=f"g_{d}_{t}")
                        nc.scalar.activation(out=gact[:], in_=ph,
                                             func=AF.Sigmoid, scale=0.0625)
                        # i:0-24 f:24-48 o:48-72 g':72-96
                        ig = p2.tile([128, 24], bf, tag=f"ig_{d}")
                        nc.vector.scalar_tensor_tensor(
                            out=ig[:], in0=gact[:, 72:96], scalar=-0.5,
                            in1=gact[:, 0:24], op0=OP.subtract, op1=OP.mult)
                        fc = p2.tile([128, 24], bf, tag=f"fc_{d}")
                        nc.vector.tensor_mul(out=fc[:], in0=gact[:, 24:48],
                                             in1=cst[d][:])
                        nc.vector.scalar_tensor_tensor(
                            out=cst[d][:], in0=ig[:], scalar=2.0, in1=fc[:],
                            op0=OP.mult, op1=OP.add)
                        tc_t = p2.tile([128, 24], bf, tag=f"tc_{d}")
                        nc.scalar.activation(out=tc_t[:], in_=cst[d][:],
                                             func=AF.Sigmoid, scale=2.0)
                        # h/2 = (sig(2c)-0.5)*o  -> hh (3-chunk layout)
                        nc.vector.scalar_tensor_tensor(
                            out=hh[d][:].rearrange("p (c x) -> p c x", c=3)[:, :, 8 * t:8 * t + 8],
                            in0=tc_t[:].rearrange("p (c x) -> p c x", c=3),
                            scalar=-0.5,
                            in1=gact[:, 48:72].rearrange("p (c x) -> p c x", c=3),
                            op0=OP.subtract, op1=OP.mult)

                    if "p2" in skip:
                        for d in "fb":
                            nc.vector.memset(hh[d][:], 0.0)
                    else:
                        # skewed pipeline: f-MMs(ss) | b-tail(ss-1) | b-MMs(ss) | f-tail(ss)
                        pend_b = None
                        for ss in range(S):
                            tf, tb = ss, S - 1 - ss
                            ph_f = step_mms("f", tf)
                            if pend_b is not None:
                                step_tail("b", pend_b[0], pend_b[1])
                            ph_b = step_mms("b", tb)
                            step_tail("f", tf, ph_f)
                            pend_b = (tb, ph_b)
                        step_tail("b", pend_b[0], pend_b[1])

                # ---------------- P3: emissions ----------------
                with tc.tile_pool(name="p3ps", bufs=4, space="PSUM") as p3ps:
                  if "p3" not in skip:
                    for n in range(0, T, 512):
                        pe = p3ps.tile([12, 512], f32, tag="pe")
                        for c in range(6):
                            hsrc = hh["f"] if c < 3 else hh["b"]
                            cc = c % 3
                            nc.tensor.matmul(
                                out=pe[:, 0:512], lhsT=plin[:, 12 * c:12 * (c + 1)],
                                rhs=hsrc[:, T * cc + n:T * cc + n + 512],
                                start=(c == 0), stop=(c == 5))
                        nc.vector.tensor_scalar(
                            out=emit[:, n:n + 512], in0=pe[:, 0:512],
                            scalar1=blin[:, 0:1], scalar2=None, op0=OP.add)
                    nc.scalar.activation(out=Ee[:], in_=emit[:], func=AF.Exp)

                # ---------------- P4: gold score ----------------
                with tc.tile_pool(name="p4", bufs=2) as p4:
                  if "p4" in skip:
                    nc.vector.memset(goldT[:], 0.0)
                  else:
                    s2 = p4.tile([12, T], bf, tag="s2")
                    with tc.tile_pool(name="p4psa", bufs=1, space="PSUM") as p4psa:
                        pts = p4psa.tile([12, T], f32, tag="pts")
                        for n in range(0, T, 512):
                            nc.tensor.matmul(out=pts[:, n:n + 512], lhsT=transT_sb[:],
                                             rhs=mask[:, 8 + n:8 + n + 512],
                                             start=True, stop=True)
                        nc.vector.tensor_add(out=s2[:], in0=pts[:], in1=emit[:])
                    nc.vector.tensor_mul(out=s2[:], in0=s2[:], in1=mask[:, 0:T])
                    with tc.tile_pool(name="p4ps", bufs=1, space="PSUM") as p4ps:
                        ps_s = p4ps.tile([1, T], f32, tag="ps_s")
                        ones12f = p4.tile([12, 1], bf, tag="onesf")
                        nc.vector.memset(ones12f[:], 1.0)
                        for n in range(0, T, 512):
                            nc.tensor.matmul(out=ps_s[:, n:n + 512], lhsT=ones12f[:],
                                             rhs=s2[:, n:n + 512], start=True, stop=True)
                        nc.vector.tensor_reduce(
                            out=goldT[:], in_=ps_s[:].rearrange("p (t b) -> p b t", b=8),
                            axis=mybir.AxisListType.X, op=OP.add)

                # ---------------- P5: pair-fused CRF scan ----------------
                nc.vector.tensor_copy(out=D[:], in_=Ee[:, 0:8])
                with tc.tile_pool(name="p5k", bufs=1) as p5k, \
                     tc.tile_pool(name="p5", bufs=4) as p5, \
                     tc.tile_pool(name="p5ps", bufs=1, space="PSUM") as p5ps:
                  if "p5" not in skip:
                    # leaves: leafT[r, j] = diag(Ee[:, 8(2r+1)+j]) @ texpT
                    leafT = p5k.tile([12, NR * 96], bf, name="leafT_sb")
                    for j in range(BC):
                        src = Ee[:, 8 + j:8 + j + NR * 16]
                        src3 = src.rearrange("p (r x) -> p r x", x=16)[:, :, 0:1]
                        nc.vector.scalar_tensor_tensor(
                            out=leafT[:].rearrange("p (r e k) -> p r e k", e=BC, k=12)[:, :, j, :],
                            in0=src3.broadcast_to([12, NR, 12]),
                            scalar=1.0, in1=texpT8_sb[:, 12 * j:12 * j + 12]
                                 .unsqueeze(1).broadcast_to([12, NR, 12]),
                            op0=OP.mult, op1=OP.mult)
                    # K'_r(j) = leafT(r,j).T @ texp  (batches of 5 rounds/bank)
                    Ksb = p5k.tile([12, NR * 96], bf, name="Ksb")
                    for r0 in range(0, NR, 5):
                        nb = min(5, NR - r0)
                        pk = p5ps.tile([12, 480], f32, tag="pk", bufs=2)
                        for r in range(r0, r0 + nb):
                            for j in range(BC):
                                nc.tensor.matmul(
                                    out=pk[:, 96 * (r - r0) + 12 * j:96 * (r - r0) + 12 * j + 12],
                                    lhsT=leafT[:, 96 * r + 12 * j:96 * r + 12 * j + 12],
                                    rhs=texp_sb[:], start=True, stop=True)
                        nc.vector.tensor_copy(out=Ksb[:, 96 * r0:96 * r0 + 96 * nb],
                                              in_=pk[:, 0:96 * nb])
                    # serial rounds: two 4-example chains
                    for r in range(NR):
                        for h in range(2):
                            pq = p5ps.tile([12, 4], f32, tag=f"pq{h}", bufs=2)
                            for j in range(4):
                                ex = 4 * h + j
                                nc.tensor.matmul(
                                    out=pq[:, j:j + 1],
                                    lhsT=Ksb[:, 96 * r + 12 * ex:96 * r + 12 * ex + 12],
                                    rhs=D[:, ex:ex + 1], start=True, stop=True)
                            nc.vector.tensor_mul(
                                out=D[:, 4 * h:4 * h + 4], in0=pq[:],
                                in1=Ee[:, 8 * (2 * r + 2) + 4 * h:8 * (2 * r + 2) + 4 * h + 4])
                    # leftover single step t = S-1 (odd tail)
                    pq0 = p5ps.tile([12, 4], f32, tag="pq0", bufs=2)
                    pq1 = p5ps.tile([12, 4], f32, tag="pq1", bufs=2)
                    nc.tensor.matmul(out=pq0[:], lhsT=texp_sb[:], rhs=D[:, 0:4],
                                     start=True, stop=True)
                    nc.tensor.matmul(out=pq1[:], lhsT=texp_sb[:], rhs=D[:, 4:8],
                                     start=True, stop=True)
                    nc.vector.tensor_mul(out=D[:, 0:4], in0=pq0[:],
                                         in1=Ee[:, 8 * (S - 1):8 * (S - 1) + 4])
                    nc.vector.tensor_mul(out=D[:, 4:8], in0=pq1[:],
                                         in1=Ee[:, 8 * (S - 1) + 4:8 * S])

                    # ---------------- P6: finalize ----------------
                    pz = p5ps.tile([1, 8], f32, tag="pz", name="pz_f")
                    nc.tensor.matmul(out=pz[:], lhsT=ones12[:], rhs=D[:],
                                     start=True, stop=True)
                    zrow = p5.tile([1, 8], f32, tag="zrow")
                    nc.scalar.activation(out=zrow[:], in_=pz[:], func=AF.Ln,
                                         bias=eps_b[0:1, 0:1])
                    nc.vector.tensor_scalar_add(out=zrow[:], in0=zrow[:],
                                                scalar1=float(DELTA * (S - 1)))
                    nc.vector.tensor_sub(out=zrow[:], in0=zrow[:], in1=goldT[:])
                    plt = p5ps.tile([8, 1], f32, tag="plt", name="plt_f")
                    nc.tensor.transpose(out=plt[0:8, 0:1], in_=zrow[:],
                                        identity=ident[0:1, 0:1])
                    nc.vector.tensor_copy(out=loss_sb[:], in_=plt[0:8, 0:1])
                  else:
                    nc.vector.memset(loss_sb[:], 0.0)
                nc.sync.dma_start(out=d_loss[:], in_=loss_sb[:])

    nc.compile()
    return nc, names


def _prepare_inputs(inputs, S):
    """Host-side packing: layout transforms only. Returns list of per-core maps."""
    from concourse import mybir
    fp8_np = mybir.dt.np(mybir.dt.float8e4)
    sent = np.asarray(inputs["sentences"]).astype(np.int32)
    tags = np.asarray(inputs["tags"]).astype(np.int32)
    trans = np.asarray(inputs["transitions"], np.float32)
    texp = np.exp(trans - DELTA)
    packed = dict(
        pxw_f=_pack_xw(np.asarray(inputs["W_ih_f"]), np.asarray(inputs["b_f"])),
        pxw_b=_pack_xw(np.asarray(inputs["W_ih_b"]), np.asarray(inputs["b_b"])),
        phw_f=_pack_hw(np.asarray(inputs["W_hh_f"]), fp8_np),
        phw_b=_pack_hw(np.asarray(inputs["W_hh_b"]), fp8_np),
        plin=_pack_lin(np.asarray(inputs["W_lin"])),
        blin=np.ascontiguousarray(np.asarray(inputs["b_lin"], np.float32)[:, None]),
        transT=_bf16(np.ascontiguousarray(trans.T)),
        texp=_bf16(texp),
        texpT8=_bf16(np.tile(np.ascontiguousarray(texp.T), (1, 8))),
        embed=np.asarray(inputs["embed_table"], np.float32),
    )
    maps = []
    for core in range(NCORES):
        sl = slice(core * BC, (core + 1) * BC)
        m = dict(packed)
        m["sent"] = np.ascontiguousarray(sent[sl, :S].T.reshape(-1))
        m["tags"] = np.ascontiguousarray(tags[sl, :S].T.reshape(-1))
        maps.append(m)
    return maps


def kernel(**inputs):
    from concourse import bass_utils
    S = 256
    if ("nc", S) not in _cache:
        _cache[("nc", S)] = build(S)
    nc, names = _cache[("nc", S)]
    maps = _prepare_inputs(inputs, S)
    in_maps = [{names[k]: v for k, v in m.items() if k != "loss"} for m in maps]
    res = bass_utils.run_bass_kernel_spmd(nc, in_maps, core_ids=list(range(NCORES)),
                                          trace=False)
    out = np.concatenate([r[names["loss"]].reshape(BC) for r in res.results])
    return out.astype(np.float32)


if __name__ == "__main__":
    import reference
    inputs = {k: np.asarray(v) for k, v in reference.setup_inputs().items()}
    expected = np.asarray(reference.reference(**inputs))
    actual = kernel(**inputs)
    rel = np.linalg.norm(actual - expected) / np.linalg.norm(expected)
    print("expected[:4]:", expected[:4])
    print("actual[:4]:  ", actual[:4])
    print("Relative error:", rel)
